# revision 11
# baseline (speedup 1.0000x reference)
"""GCN encoder (GIN conv -> 2x GCN conv) on 8 Trainium2 NeuronCores.

Strategy (dst-sharded, graph-parallel, fp8-e3m4 message streams):
- Nodes sharded by dst across 8 cores (12500 each); each core owns the
  segment-sums and dense math for its nodes; weights replicated.
- Self-loops ride the edge stream as synthetic (i, i) edges.
- Slot rows are float8 e3m4 with one global scale (absmax ~ 15); the scale
  is undone by host-prescaled weights (launch A) / an ACT scale operand
  (launch C). GCN dinv_i*dinv_j normalization is folded into the host
  gather that builds launch C's rows.
- Launch A (needs feature-major activations for the two dense layers):
  per-block slot layout [d_b, 128, 64]; aggregation via TensorE
  transpose-accumulate (lhsT = slot pair, rhs = identity) into PSUM;
  the two 64-row halves are summed for free by a row-duplicated [W;W]
  weight in the following GIN matmul.
- Launch C (elementwise epilogue only): per-SUPERTILE slot layout
  [d_st, 128, 4*2*64]; aggregation via wide matmuls (lhsT = identity,
  rhs = 512-col slice covering 4 blocks' slot pairs) accumulating
  node-major in PSUM -- 4 slot tiles per matmul instead of 2, and no
  transpose or reduction matmul afterwards. Output is written node-major
  [128, NBLK*64]; the host unshards it.

Two SPMD launches, host gather between them (p table -> C slot layout
with dinv_i*dinv_j/s2 applied during the gather).
"""

import numpy as np
import ml_dtypes

BF16 = ml_dtypes.bfloat16
E3M4 = ml_dtypes.float8_e3m4

N = 100000
E = 1600000
COUT = 32
NCORES = 8
NPC = N // NCORES            # 12500 real nodes per core
BLK = 128
NBLK = 100                   # blocks per core
SB = 4                       # blocks per supertile (shares one PSUM bank)
NST = NBLK // SB             # 25 supertiles
GRPB = 8                     # blocks per slot DMA (2 supertiles)
NPCP = NBLK * BLK            # 12800 padded positions per core
AMAX = 15.0                  # e3m4 target absmax (max normal 15.5)

_cache = {}


def _build_A(d_sched):
    import concourse.bacc as bacc
    import concourse.mybir as mybir
    import concourse.tile as tile

    tile_off = np.concatenate([[0], np.cumsum(d_sched)]).astype(int)
    t1 = int(tile_off[-1])
    gd8 = max(int(tile_off[min(g + GRPB, NBLK)] - tile_off[g])
              for g in range(0, NBLK, GRPB))

    nc = bacc.Bacc("TRN2", target_bir_lowering=False, debug=False,
                   enable_asserts=False, num_devices=NCORES)
    slots = nc.dram_tensor("slots", [BLK, t1, 64], mybir.dt.float8e3,
                           kind="ExternalInput").ap()
    identD = nc.dram_tensor("identD", [BLK, BLK], mybir.dt.float8e3,
                            kind="ExternalInput").ap()
    ginW = nc.dram_tensor("ginW", [128, 64], mybir.dt.bfloat16,
                          kind="ExternalInput").ap()
    ginb = nc.dram_tensor("ginb", [64, 1], mybir.dt.float32,
                          kind="ExternalInput").ap()
    wcat = nc.dram_tensor("wcat", [64, 64], mybir.dt.bfloat16,
                          kind="ExternalInput").ap()
    outT = nc.dram_tensor("outT", [64, NPCP], mybir.dt.bfloat16,
                          kind="ExternalOutput").ap()

    with tile.TileContext(nc) as tc:
        with (tc.tile_pool(name="const", bufs=1) as cpool,
              tc.tile_pool(name="blkin", bufs=4) as bpool,
              tc.tile_pool(name="work", bufs=4) as wpool,
              tc.tile_pool(name="ps", bufs=4, space="PSUM") as ppool,
              tc.tile_pool(name="ps2", bufs=2, space="PSUM") as p2pool):
            ident = cpool.tile([BLK, BLK], mybir.dt.float8e3)
            nc.scalar.dma_start(out=ident[:], in_=identD[:])
            ginW_sb = cpool.tile([128, 64], mybir.dt.bfloat16)
            nc.scalar.dma_start(out=ginW_sb[:], in_=ginW[:])
            ginb_sb = cpool.tile([64, 1], mybir.dt.float32)
            nc.scalar.dma_start(out=ginb_sb[:], in_=ginb[:])
            wcat_sb = cpool.tile([64, 64], mybir.dt.bfloat16)
            nc.scalar.dma_start(out=wcat_sb[:], in_=wcat[:])

            W = SB * BLK
            for g0 in reversed(range(0, NBLK, GRPB)):
                nb = min(GRPB, NBLK - g0)
                gt0 = int(tile_off[g0])
                gtn = int(tile_off[g0 + nb] - gt0)
                blkt = bpool.tile([BLK, gd8 * 64], mybir.dt.float8e3,
                                  tag="blk")
                nc.sync.dma_start(out=blkt[:, :gtn * 64],
                                  in_=slots[:, gt0:gt0 + gtn, :])
                otg = None
                for si in range(nb // SB):
                    b0 = g0 + si * SB
                    ps = ppool.tile([BLK, W], mybir.dt.float32, space="PSUM")
                    for j in range(SB):
                        b = b0 + j
                        db = int(d_sched[b])
                        o = int(tile_off[b] - gt0)
                        for s in range(db // 2):
                            nc.tensor.matmul(
                                out=ps[:, j * BLK:(j + 1) * BLK],
                                lhsT=blkt[:, (o + s * 2) * 64:
                                          (o + s * 2 + 2) * 64],
                                rhs=ident[:],
                                start=(s == 0),
                                stop=(s == db // 2 - 1),
                            )
                    xin = wpool.tile([BLK, W], mybir.dt.bfloat16, tag="xin")
                    nc.vector.tensor_scalar_mul(xin[:], ps[:], 1.0)
                    ps2 = p2pool.tile([64, W], mybir.dt.float32, space="PSUM")
                    nc.tensor.matmul(out=ps2[:], lhsT=ginW_sb[:], rhs=xin[:],
                                     start=True, stop=True)
                    hT = wpool.tile([64, W], mybir.dt.bfloat16, tag="hT")
                    nc.scalar.activation(hT[:], ps2[:],
                                         mybir.ActivationFunctionType.Relu,
                                         bias=ginb_sb[:], scale=1.0)
                    ps3 = p2pool.tile([64, W], mybir.dt.float32, space="PSUM")
                    nc.tensor.matmul(out=ps3[:], lhsT=wcat_sb[:], rhs=hT[:],
                                     start=True, stop=True)
                    if si == 0:
                        otg = wpool.tile([64, (nb // SB) * W],
                                         mybir.dt.bfloat16, tag="ot")
                    nc.vector.tensor_scalar_mul(
                        otg[:, si * W:(si + 1) * W], ps3[:], 1.0)
                    if si == nb // SB - 1:
                        nc.scalar.dma_start(
                            out=outT[:, g0 * BLK:(g0 + nb) * BLK],
                            in_=otg[:, :(nb // SB) * W])
    nc.compile()
    from concourse.bass_interp import get_hw_module
    nc.m = get_hw_module(nc.m)
    return nc


def _build_C(d_st, has_bias):
    import concourse.bacc as bacc
    import concourse.mybir as mybir
    import concourse.tile as tile

    # supertile st occupies d_st[st]*4 tiles of 64 cols (d/2 pairs x 8)
    st_off = np.concatenate([[0], np.cumsum(d_st * 4)]).astype(int)
    t1c = int(st_off[-1])                    # total 64-col tiles
    GS = GRPB // SB                          # supertiles per DMA group (2)
    gdc = max(int(st_off[min(g + GS, NST)] - st_off[g])
              for g in range(0, NST, GS))

    nc = bacc.Bacc("TRN2", target_bir_lowering=False, debug=False,
                   enable_asserts=False, num_devices=NCORES)
    slots = nc.dram_tensor("slots", [BLK, t1c, 64], mybir.dt.float8e3,
                           kind="ExternalInput").ap()
    identD = nc.dram_tensor("identD", [BLK, BLK], mybir.dt.float8e3,
                            kind="ExternalInput").ap()
    scl = nc.dram_tensor("scl", [BLK, 1], mybir.dt.float32,
                         kind="ExternalInput").ap()
    if has_bias:
        biasT = nc.dram_tensor("biasT", [BLK, SB * 64], mybir.dt.float32,
                               kind="ExternalInput").ap()
    outT = nc.dram_tensor("outT", [BLK, NBLK * 64], mybir.dt.bfloat16,
                          kind="ExternalOutput").ap()

    with tile.TileContext(nc) as tc:
        with (tc.tile_pool(name="const", bufs=1) as cpool,
              tc.tile_pool(name="blkin", bufs=4) as bpool,
              tc.tile_pool(name="work", bufs=4) as wpool,
              tc.tile_pool(name="ps", bufs=6, space="PSUM") as ppool):
            ident = cpool.tile([BLK, BLK], mybir.dt.float8e3)
            nc.scalar.dma_start(out=ident[:], in_=identD[:])
            scl_sb = cpool.tile([BLK, 1], mybir.dt.float32)
            nc.scalar.dma_start(out=scl_sb[:], in_=scl[:])
            if has_bias:
                bias_sb = cpool.tile([BLK, SB * 64], mybir.dt.float32)
                nc.scalar.dma_start(out=bias_sb[:], in_=biasT[:])

            W = SB * BLK                     # psum width (512)
            HW_ = SB * 64                    # epilogue width (256)
            for g0 in reversed(range(0, NST, GS)):
                ns = min(GS, NST - g0)
                gt0 = int(st_off[g0])
                gtn = int(st_off[g0 + ns] - gt0)
                blkt = bpool.tile([BLK, gdc * 64], mybir.dt.float8e3,
                                  tag="blk")
                nc.sync.dma_start(out=blkt[:, :gtn * 64],
                                  in_=slots[:, gt0:gt0 + gtn, :])
                otg = None
                for si in range(ns):
                    st = g0 + si
                    o = int(st_off[st] - gt0) * 64
                    dh = int(d_st[st]) // 2
                    ps = ppool.tile([BLK, W], mybir.dt.float32, space="PSUM")
                    for s in range(dh):
                        nc.tensor.matmul(
                            out=ps[:],
                            lhsT=ident[:],
                            rhs=blkt[:, o + s * 512: o + (s + 1) * 512],
                            start=(s == 0),
                            stop=(s == dh - 1),
                        )
                    psv = ps[:].rearrange("p (j t f) -> p j t f",
                                          j=SB, t=2, f=64)
                    a = wpool.tile([BLK, HW_], mybir.dt.float32, tag="a")
                    nc.vector.tensor_scalar_mul(
                        a[:].rearrange("p (j f) -> p j f", j=SB, f=64),
                        psv[:, :, 0, :], 1.0)
                    sm = wpool.tile([BLK, HW_], mybir.dt.float32, tag="sm")
                    nc.vector.tensor_add(
                        out=sm[:].rearrange("p (j f) -> p j f", j=SB, f=64),
                        in0=a[:].rearrange("p (j f) -> p j f", j=SB, f=64),
                        in1=psv[:, :, 1, :])
                    if has_bias:
                        nc.vector.tensor_add(out=sm[:], in0=sm[:],
                                             in1=bias_sb[:])
                    if si == 0:
                        otg = wpool.tile([BLK, ns * HW_], mybir.dt.bfloat16,
                                         tag="ot")
                    osl = slice(si * HW_, (si + 1) * HW_)
                    nc.scalar.activation(otg[:, osl], sm[:],
                                         mybir.ActivationFunctionType.Identity,
                                         bias=0.0, scale=scl_sb[:])
                    muv = otg[:, osl].rearrange("p (j f) -> p j f",
                                                j=SB, f=64)[:, :, 0:COUT]
                    nc.vector.tensor_scalar_max(muv, muv, 0.0)
                    if si == ns - 1:
                        nc.scalar.dma_start(
                            out=outT[:, g0 * HW_:(g0 + ns) * HW_],
                            in_=otg[:, :ns * HW_])
    nc.compile()
    from concourse.bass_interp import get_hw_module
    nc.m = get_hw_module(nc.m)
    return nc


def _prep(edge_index):
    """Shard/sort/pad the graph (self-loops appended as real edges)."""
    src0 = np.asarray(edge_index[0], dtype=np.int64)
    dst0 = np.asarray(edge_index[1], dtype=np.int64)
    deg_in = np.bincount(dst0, minlength=N)
    dinv = (1.0 / np.sqrt(deg_in + 1.0)).astype(np.float32)
    allN = np.arange(N, dtype=np.int64)
    src = np.concatenate([src0, allN])
    dst = np.concatenate([dst0, allN])

    cores = []
    d_sched_per_core = np.zeros((NCORES, NBLK), dtype=np.int64)
    for c in range(NCORES):
        lo, hi = c * NPC, (c + 1) * NPC
        m = (dst >= lo) & (dst < hi)
        s_c = src[m]
        d_c = (dst[m] - lo).astype(np.int64)
        deg_c = np.bincount(d_c, minlength=NPC)
        order = np.argsort(deg_c, kind="stable")      # position -> local node
        pos = np.empty(NPC, dtype=np.int64)
        pos[order] = np.arange(NPC)                   # local node -> position
        posdeg = np.zeros(NPCP, dtype=np.int64)
        posdeg[:NPC] = deg_c[order]
        d_sched_per_core[c] = posdeg.reshape(NBLK, BLK).max(axis=1)
        cores.append((s_c, d_c, order, pos, posdeg))

    d_sched = d_sched_per_core.max(axis=0)
    d_sched = np.maximum(d_sched, 2)
    d_sched = ((d_sched + 1) // 2) * 2        # even: paired matmuls
    tile_off = np.concatenate([[0], np.cumsum(d_sched)]).astype(np.int64)
    t1 = int(tile_off[-1])
    d_st = d_sched.reshape(NST, SB).max(axis=1)       # supertile pad for C
    st_off = np.concatenate([[0], np.cumsum(d_st * 4)]).astype(np.int64)
    t1c = int(st_off[-1])

    srcidx = np.full((NCORES, t1, BLK), -1, dtype=np.int64)    # launch A
    srcidxC = np.full((NCORES, t1c, BLK), -1, dtype=np.int64)  # launch C
    coefC = np.zeros((NCORES, t1c, BLK), dtype=np.float32)
    pos_of_global = np.empty(N, dtype=np.int64)
    for c in range(NCORES):
        s_c, d_c, order, pos, posdeg = cores[c]
        pos_of_global[c * NPC + order] = c * NPCP + np.arange(NPC)
        key = pos[d_c]
        eord = np.argsort(key, kind="stable")
        spos = key[eord]                              # node position per edge
        start_of_pos = np.zeros(NPCP, dtype=np.int64)
        np.cumsum(posdeg[:-1], out=start_of_pos[1:])
        r = np.arange(len(spos)) - start_of_pos[spos]  # rank within node
        se = s_c[eord]
        de = d_c[eord] + c * NPC
        blk = spos // BLK
        prow = spos % BLK
        # A layout: block-padded, tile index t in [0, t1)
        tA = tile_off[blk] + r
        srcidx[c, tA, prow] = se
        # C layout: supertile-padded; 64-col tile index =
        #   st_off[st] + (r//2)*8 + (block-within-supertile)*2 + parity
        stb = blk // SB
        jj = blk % SB
        tC = st_off[stb] + (r // 2) * 8 + jj * 2 + (r % 2)
        srcidxC[c, tC, prow] = se
        coefC[c, tC, prow] = dinv[se] * dinv[de]
    return (d_sched, t1, srcidx, d_st, t1c, srcidxC, coefC,
            pos_of_global, dinv, cores)


TRACE = False
last_exec_ns = []


def _run(nc, in_maps):
    from concourse import bass_utils
    res = bass_utils.run_bass_kernel_spmd(nc, in_maps,
                                          core_ids=list(range(NCORES)),
                                          trace=TRACE)
    if TRACE:
        last_exec_ns.append(res.exec_time_ns)
    return res.results


def kernel(x, edge_index, gin_W, gin_b, mu_W, mu_b, lv_W, lv_b):
    x = np.asarray(x, dtype=np.float32)
    gin_W = np.asarray(gin_W, dtype=np.float32)
    gin_b = np.asarray(gin_b, dtype=np.float32)
    wcat = np.concatenate([np.asarray(mu_W, np.float32),
                           np.asarray(lv_W, np.float32)], axis=1)
    bias_cat = np.concatenate([np.asarray(mu_b, np.float32),
                               np.asarray(lv_b, np.float32)])
    has_bias = bool(np.any(bias_cat != 0))

    (d_sched, t1, srcidx, d_st, t1c, srcidxC, coefC,
     pos_of_global, dinv, cores) = _prep(edge_index)

    key = ("prog", t1, t1c, has_bias, tuple(int(v) for v in d_sched))
    if key not in _cache:
        _cache[key] = (_build_A(d_sched), _build_C(d_st, has_bias))
    nc_A, nc_C = _cache[key]

    identM = np.eye(BLK, dtype=np.float32).astype(E3M4)

    # ---- launch A inputs ----
    s1 = float(np.abs(x).max()) / AMAX
    xq = (x / s1).astype(E3M4)
    x_pad = np.zeros((N + 1, 64), dtype=E3M4)
    x_pad[:N] = xq
    gather1 = np.where(srcidx >= 0, srcidx, N)

    in_maps_A = []
    for c in range(NCORES):
        in_maps_A.append({
            "slots": np.ascontiguousarray(
                x_pad[gather1[c]].transpose(1, 0, 2)),
            "identD": identM,
            "ginW": np.vstack([s1 * gin_W, s1 * gin_W]).astype(BF16),
            "ginb": gin_b.reshape(64, 1),
            "wcat": wcat.astype(BF16),
        })
    res_A = _run(nc_A, in_maps_A)

    # ---- assemble p table, build launch C inputs ----
    p_pos = np.zeros((NCORES * NPCP + 1, 64), dtype=np.float32)
    for c in range(NCORES):
        p_pos[c * NPCP:(c + 1) * NPCP] = res_A[c]["outT"].T
    gather2 = np.where(srcidxC >= 0, pos_of_global[srcidxC],
                       NCORES * NPCP)

    rowmax = np.abs(p_pos).max(axis=1)
    s2 = 0.0
    for c in range(NCORES):
        s2 = max(s2, float((coefC[c] * rowmax[gather2[c]]).max()))
    s2 /= AMAX

    in_maps_C = []
    for c in range(NCORES):
        vals = p_pos[gather2[c]] * (coefC[c] / s2)[:, :, None]
        im = {
            "slots": np.ascontiguousarray(
                vals.astype(E3M4).transpose(1, 0, 2)),
            "identD": identM,
            "scl": np.full((BLK, 1), s2, dtype=np.float32),
        }
        if has_bias:
            im["biasT"] = np.broadcast_to(
                np.tile(bias_cat, SB)[None, :], (BLK, SB * 64)
            ).astype(np.float32).copy()
        in_maps_C.append(im)
    res_C = _run(nc_C, in_maps_C)

    # ---- unshard (C output is node-major [128, NBLK*64]) ----
    out = np.empty((N, 64), dtype=np.float32)
    for c in range(NCORES):
        _, _, order, _, _ = cores[c]
        o = res_C[c]["outT"].reshape(BLK, NBLK, 64).transpose(1, 0, 2)
        out[c * NPC + order] = o.reshape(NPCP, 64)[:NPC]
    return out[:, :COUT], out[:, COUT:]


# revision 12
# speedup vs baseline: 1.0785x; 1.0785x over previous
"""GCN encoder (GIN conv -> 2x GCN conv) on 8 Trainium2 NeuronCores.

Strategy (dst-sharded, graph-parallel, fp8-e3m4 message streams):
- Nodes sharded by dst across 8 cores (12500 each); each core owns the
  segment-sums and dense math for its nodes; weights replicated.
- Self-loops ride the edge stream as synthetic (i, i) edges.
- Slot rows are float8 e3m4 with one global scale (absmax ~ 15); the scale
  is undone by host-prescaled weights (launch A) / an ACT scale operand
  (launch C). GCN dinv_i*dinv_j normalization is folded into the host
  gather that builds launch C's rows.
- Launch A (needs feature-major activations for the two dense layers):
  per-block slot layout [d_b, 128, 64]; aggregation via TensorE
  transpose-accumulate (lhsT = slot pair, rhs = identity) into PSUM;
  the two 64-row halves are summed for free by a row-duplicated [W;W]
  weight in the following GIN matmul.
- Launch C (elementwise epilogue only): per-SUPERTILE slot layout
  [d_st, 128, 4*2*64]; aggregation via wide matmuls (lhsT = identity,
  rhs = 512-col slice covering 4 blocks' slot pairs) accumulating
  node-major in PSUM -- 4 slot tiles per matmul instead of 2, and no
  transpose or reduction matmul afterwards. Output is written node-major
  [128, NBLK*64]; the host unshards it.

Two SPMD launches, host gather between them (p table -> C slot layout
with dinv_i*dinv_j/s2 applied during the gather).
"""

import numpy as np
import ml_dtypes

BF16 = ml_dtypes.bfloat16
E3M4 = ml_dtypes.float8_e3m4

N = 100000
E = 1600000
COUT = 32
NCORES = 8
NPC = N // NCORES            # 12500 real nodes per core
BLK = 128
NBLK = 100                   # blocks per core
SB = 4                       # blocks per supertile (shares one PSUM bank)
NST = NBLK // SB             # 25 supertiles
GRPB = 8                     # blocks per slot DMA (2 supertiles)
NPCP = NBLK * BLK            # 12800 padded positions per core
AMAX = 15.0                  # e3m4 target absmax (max normal 15.5)

_cache = {}


def _build_A(d_sched):
    import concourse.bacc as bacc
    import concourse.mybir as mybir
    import concourse.tile as tile

    tile_off = np.concatenate([[0], np.cumsum(d_sched)]).astype(int)
    t1 = int(tile_off[-1])
    gd8 = max(int(tile_off[min(g + GRPB, NBLK)] - tile_off[g])
              for g in range(0, NBLK, GRPB))

    nc = bacc.Bacc("TRN2", target_bir_lowering=False, debug=False,
                   enable_asserts=False, num_devices=NCORES)
    slots = nc.dram_tensor("slots", [BLK, t1, 64], mybir.dt.float8e3,
                           kind="ExternalInput").ap()
    identD = nc.dram_tensor("identD", [BLK, BLK], mybir.dt.float8e3,
                            kind="ExternalInput").ap()
    ginW = nc.dram_tensor("ginW", [128, 64], mybir.dt.bfloat16,
                          kind="ExternalInput").ap()
    ginb = nc.dram_tensor("ginb", [64, 1], mybir.dt.float32,
                          kind="ExternalInput").ap()
    wcat = nc.dram_tensor("wcat", [64, 64], mybir.dt.bfloat16,
                          kind="ExternalInput").ap()
    outT = nc.dram_tensor("outT", [64, NPCP], mybir.dt.bfloat16,
                          kind="ExternalOutput").ap()

    with tile.TileContext(nc) as tc:
        with (tc.tile_pool(name="const", bufs=1) as cpool,
              tc.tile_pool(name="blkin", bufs=4) as bpool,
              tc.tile_pool(name="work", bufs=4) as wpool,
              tc.tile_pool(name="ps", bufs=4, space="PSUM") as ppool,
              tc.tile_pool(name="ps2", bufs=2, space="PSUM") as p2pool):
            ident = cpool.tile([BLK, BLK], mybir.dt.float8e3)
            nc.scalar.dma_start(out=ident[:], in_=identD[:])
            ginW_sb = cpool.tile([128, 64], mybir.dt.bfloat16)
            nc.scalar.dma_start(out=ginW_sb[:], in_=ginW[:])
            ginb_sb = cpool.tile([64, 1], mybir.dt.float32)
            nc.scalar.dma_start(out=ginb_sb[:], in_=ginb[:])
            wcat_sb = cpool.tile([64, 64], mybir.dt.bfloat16)
            nc.scalar.dma_start(out=wcat_sb[:], in_=wcat[:])

            W = SB * BLK
            groups = [(0, SB), (SB, SB)] + [
                (g, min(GRPB, NBLK - g)) for g in range(GRPB, NBLK, GRPB)]
            for g0, nb in groups:
                gt0 = int(tile_off[g0])
                gtn = int(tile_off[g0 + nb] - gt0)
                blkt = bpool.tile([BLK, gd8 * 64], mybir.dt.float8e3,
                                  tag="blk")
                nc.sync.dma_start(out=blkt[:, :gtn * 64],
                                  in_=slots[:, gt0:gt0 + gtn, :])
                otg = None
                for si in range(nb // SB):
                    b0 = g0 + si * SB
                    ps = ppool.tile([BLK, W], mybir.dt.float32, space="PSUM")
                    for j in range(SB):
                        b = b0 + j
                        db = int(d_sched[b])
                        o = int(tile_off[b] - gt0)
                        for s in range(db // 2):
                            nc.tensor.matmul(
                                out=ps[:, j * BLK:(j + 1) * BLK],
                                lhsT=blkt[:, (o + s * 2) * 64:
                                          (o + s * 2 + 2) * 64],
                                rhs=ident[:],
                                start=(s == 0),
                                stop=(s == db // 2 - 1),
                            )
                    xin = wpool.tile([BLK, W], mybir.dt.bfloat16, tag="xin")
                    nc.vector.tensor_scalar_mul(xin[:], ps[:], 1.0)
                    ps2 = p2pool.tile([64, W], mybir.dt.float32, space="PSUM")
                    nc.tensor.matmul(out=ps2[:], lhsT=ginW_sb[:], rhs=xin[:],
                                     start=True, stop=True)
                    hT = wpool.tile([64, W], mybir.dt.bfloat16, tag="hT")
                    nc.scalar.activation(hT[:], ps2[:],
                                         mybir.ActivationFunctionType.Relu,
                                         bias=ginb_sb[:], scale=1.0)
                    ps3 = p2pool.tile([64, W], mybir.dt.float32, space="PSUM")
                    nc.tensor.matmul(out=ps3[:], lhsT=wcat_sb[:], rhs=hT[:],
                                     start=True, stop=True)
                    if si == 0:
                        otg = wpool.tile([64, (nb // SB) * W],
                                         mybir.dt.bfloat16, tag="ot")
                    nc.vector.tensor_scalar_mul(
                        otg[:, si * W:(si + 1) * W], ps3[:], 1.0)
                    if si == nb // SB - 1:
                        nc.scalar.dma_start(
                            out=outT[:, g0 * BLK:(g0 + nb) * BLK],
                            in_=otg[:, :(nb // SB) * W])
    nc.compile()
    from concourse.bass_interp import get_hw_module
    nc.m = get_hw_module(nc.m)
    return nc


def _build_C(d_st, has_bias):
    import concourse.bacc as bacc
    import concourse.mybir as mybir
    import concourse.tile as tile

    # supertile st occupies d_st[st]*4 tiles of 64 cols (d/2 pairs x 8)
    st_off = np.concatenate([[0], np.cumsum(d_st * 4)]).astype(int)
    t1c = int(st_off[-1])                    # total 64-col tiles
    GS = GRPB // SB                          # supertiles per DMA group (2)
    gdc = max(int(st_off[min(g + GS, NST)] - st_off[g])
              for g in range(0, NST, GS))

    nc = bacc.Bacc("TRN2", target_bir_lowering=False, debug=False,
                   enable_asserts=False, num_devices=NCORES)
    slots = nc.dram_tensor("slots", [BLK, t1c, 64], mybir.dt.float8e3,
                           kind="ExternalInput").ap()
    identD = nc.dram_tensor("identD", [BLK, BLK], mybir.dt.float8e3,
                            kind="ExternalInput").ap()
    scl = nc.dram_tensor("scl", [BLK, 1], mybir.dt.float32,
                         kind="ExternalInput").ap()
    if has_bias:
        biasT = nc.dram_tensor("biasT", [BLK, SB * 64], mybir.dt.float32,
                               kind="ExternalInput").ap()
    outT = nc.dram_tensor("outT", [BLK, NBLK * 64], mybir.dt.bfloat16,
                          kind="ExternalOutput").ap()

    with tile.TileContext(nc) as tc:
        with (tc.tile_pool(name="const", bufs=1) as cpool,
              tc.tile_pool(name="blkin", bufs=4) as bpool,
              tc.tile_pool(name="work", bufs=4) as wpool,
              tc.tile_pool(name="ps", bufs=6, space="PSUM") as ppool):
            ident = cpool.tile([BLK, BLK], mybir.dt.float8e3)
            nc.scalar.dma_start(out=ident[:], in_=identD[:])
            scl_sb = cpool.tile([BLK, 1], mybir.dt.float32)
            nc.scalar.dma_start(out=scl_sb[:], in_=scl[:])
            if has_bias:
                bias_sb = cpool.tile([BLK, SB * 64], mybir.dt.float32)
                nc.scalar.dma_start(out=bias_sb[:], in_=biasT[:])

            W = SB * BLK                     # psum width (512)
            HW_ = SB * 64                    # epilogue width (256)
            groups = [(0, 1), (1, 1)] + [
                (g, min(GS, NST - g)) for g in range(GS, NST, GS)]
            for g0, ns in groups:
                gt0 = int(st_off[g0])
                gtn = int(st_off[g0 + ns] - gt0)
                blkt = bpool.tile([BLK, gdc * 64], mybir.dt.float8e3,
                                  tag="blk")
                nc.sync.dma_start(out=blkt[:, :gtn * 64],
                                  in_=slots[:, gt0:gt0 + gtn, :])
                otg = None
                for si in range(ns):
                    st = g0 + si
                    o = int(st_off[st] - gt0) * 64
                    dh = int(d_st[st]) // 2
                    ps = ppool.tile([BLK, W], mybir.dt.float32, space="PSUM")
                    for s in range(dh):
                        nc.tensor.matmul(
                            out=ps[:],
                            lhsT=ident[:],
                            rhs=blkt[:, o + s * 512: o + (s + 1) * 512],
                            start=(s == 0),
                            stop=(s == dh - 1),
                        )
                    psv = ps[:].rearrange("p (j t f) -> p j t f",
                                          j=SB, t=2, f=64)
                    a = wpool.tile([BLK, HW_], mybir.dt.float32, tag="a")
                    nc.vector.tensor_scalar_mul(
                        a[:].rearrange("p (j f) -> p j f", j=SB, f=64),
                        psv[:, :, 0, :], 1.0)
                    sm = wpool.tile([BLK, HW_], mybir.dt.float32, tag="sm")
                    nc.vector.tensor_add(
                        out=sm[:].rearrange("p (j f) -> p j f", j=SB, f=64),
                        in0=a[:].rearrange("p (j f) -> p j f", j=SB, f=64),
                        in1=psv[:, :, 1, :])
                    if has_bias:
                        nc.vector.tensor_add(out=sm[:], in0=sm[:],
                                             in1=bias_sb[:])
                    if si == 0:
                        otg = wpool.tile([BLK, ns * HW_], mybir.dt.bfloat16,
                                         tag="ot")
                    osl = slice(si * HW_, (si + 1) * HW_)
                    nc.scalar.activation(otg[:, osl], sm[:],
                                         mybir.ActivationFunctionType.Identity,
                                         bias=0.0, scale=scl_sb[:])
                    muv = otg[:, osl].rearrange("p (j f) -> p j f",
                                                j=SB, f=64)[:, :, 0:COUT]
                    nc.vector.tensor_scalar_max(muv, muv, 0.0)
                    if si == ns - 1:
                        nc.scalar.dma_start(
                            out=outT[:, g0 * HW_:(g0 + ns) * HW_],
                            in_=otg[:, :ns * HW_])
    nc.compile()
    from concourse.bass_interp import get_hw_module
    nc.m = get_hw_module(nc.m)
    return nc


def _prep(edge_index):
    """Shard/sort/pad the graph (self-loops appended as real edges)."""
    src0 = np.asarray(edge_index[0], dtype=np.int64)
    dst0 = np.asarray(edge_index[1], dtype=np.int64)
    deg_in = np.bincount(dst0, minlength=N)
    dinv = (1.0 / np.sqrt(deg_in + 1.0)).astype(np.float32)
    allN = np.arange(N, dtype=np.int64)
    src = np.concatenate([src0, allN])
    dst = np.concatenate([dst0, allN])

    cores = []
    d_sched_per_core = np.zeros((NCORES, NBLK), dtype=np.int64)
    for c in range(NCORES):
        lo, hi = c * NPC, (c + 1) * NPC
        m = (dst >= lo) & (dst < hi)
        s_c = src[m]
        d_c = (dst[m] - lo).astype(np.int64)
        deg_c = np.bincount(d_c, minlength=NPC)
        order = np.argsort(deg_c, kind="stable")      # position -> local node
        pos = np.empty(NPC, dtype=np.int64)
        pos[order] = np.arange(NPC)                   # local node -> position
        posdeg = np.zeros(NPCP, dtype=np.int64)
        posdeg[:NPC] = deg_c[order]
        d_sched_per_core[c] = posdeg.reshape(NBLK, BLK).max(axis=1)
        cores.append((s_c, d_c, order, pos, posdeg))

    d_sched = d_sched_per_core.max(axis=0)
    d_sched = np.maximum(d_sched, 2)
    d_sched = ((d_sched + 1) // 2) * 2        # even: paired matmuls
    tile_off = np.concatenate([[0], np.cumsum(d_sched)]).astype(np.int64)
    t1 = int(tile_off[-1])
    d_st = d_sched.reshape(NST, SB).max(axis=1)       # supertile pad for C
    st_off = np.concatenate([[0], np.cumsum(d_st * 4)]).astype(np.int64)
    t1c = int(st_off[-1])

    srcidx = np.full((NCORES, t1, BLK), -1, dtype=np.int64)    # launch A
    srcidxC = np.full((NCORES, t1c, BLK), -1, dtype=np.int64)  # launch C
    coefC = np.zeros((NCORES, t1c, BLK), dtype=np.float32)
    pos_of_global = np.empty(N, dtype=np.int64)
    for c in range(NCORES):
        s_c, d_c, order, pos, posdeg = cores[c]
        pos_of_global[c * NPC + order] = c * NPCP + np.arange(NPC)
        key = pos[d_c]
        eord = np.argsort(key, kind="stable")
        spos = key[eord]                              # node position per edge
        start_of_pos = np.zeros(NPCP, dtype=np.int64)
        np.cumsum(posdeg[:-1], out=start_of_pos[1:])
        r = np.arange(len(spos)) - start_of_pos[spos]  # rank within node
        se = s_c[eord]
        de = d_c[eord] + c * NPC
        blk = spos // BLK
        prow = spos % BLK
        # A layout: block-padded, tile index t in [0, t1)
        tA = tile_off[blk] + r
        srcidx[c, tA, prow] = se
        # C layout: supertile-padded; 64-col tile index =
        #   st_off[st] + (r//2)*8 + (block-within-supertile)*2 + parity
        stb = blk // SB
        jj = blk % SB
        tC = st_off[stb] + (r // 2) * 8 + jj * 2 + (r % 2)
        srcidxC[c, tC, prow] = se
        coefC[c, tC, prow] = dinv[se] * dinv[de]
    return (d_sched, t1, srcidx, d_st, t1c, srcidxC, coefC,
            pos_of_global, dinv, cores)


TRACE = False
last_exec_ns = []


def _run(nc, in_maps):
    from concourse import bass_utils
    res = bass_utils.run_bass_kernel_spmd(nc, in_maps,
                                          core_ids=list(range(NCORES)),
                                          trace=TRACE)
    if TRACE:
        last_exec_ns.append(res.exec_time_ns)
    return res.results


def kernel(x, edge_index, gin_W, gin_b, mu_W, mu_b, lv_W, lv_b):
    x = np.asarray(x, dtype=np.float32)
    gin_W = np.asarray(gin_W, dtype=np.float32)
    gin_b = np.asarray(gin_b, dtype=np.float32)
    wcat = np.concatenate([np.asarray(mu_W, np.float32),
                           np.asarray(lv_W, np.float32)], axis=1)
    bias_cat = np.concatenate([np.asarray(mu_b, np.float32),
                               np.asarray(lv_b, np.float32)])
    has_bias = bool(np.any(bias_cat != 0))

    (d_sched, t1, srcidx, d_st, t1c, srcidxC, coefC,
     pos_of_global, dinv, cores) = _prep(edge_index)

    key = ("prog", t1, t1c, has_bias, tuple(int(v) for v in d_sched))
    if key not in _cache:
        _cache[key] = (_build_A(d_sched), _build_C(d_st, has_bias))
    nc_A, nc_C = _cache[key]

    identM = np.eye(BLK, dtype=np.float32).astype(E3M4)

    # ---- launch A inputs ----
    s1 = float(np.abs(x).max()) / AMAX
    xq = (x / s1).astype(E3M4)
    x_pad = np.zeros((N + 1, 64), dtype=E3M4)
    x_pad[:N] = xq
    gather1 = np.where(srcidx >= 0, srcidx, N)

    in_maps_A = []
    for c in range(NCORES):
        in_maps_A.append({
            "slots": np.ascontiguousarray(
                x_pad[gather1[c]].transpose(1, 0, 2)),
            "identD": identM,
            "ginW": np.vstack([s1 * gin_W, s1 * gin_W]).astype(BF16),
            "ginb": gin_b.reshape(64, 1),
            "wcat": wcat.astype(BF16),
        })
    res_A = _run(nc_A, in_maps_A)

    # ---- assemble p table, build launch C inputs ----
    p_pos = np.zeros((NCORES * NPCP + 1, 64), dtype=np.float32)
    for c in range(NCORES):
        p_pos[c * NPCP:(c + 1) * NPCP] = res_A[c]["outT"].T
    gather2 = np.where(srcidxC >= 0, pos_of_global[srcidxC],
                       NCORES * NPCP)

    rowmax = np.abs(p_pos).max(axis=1)
    s2 = 0.0
    for c in range(NCORES):
        s2 = max(s2, float((coefC[c] * rowmax[gather2[c]]).max()))
    s2 /= AMAX

    in_maps_C = []
    for c in range(NCORES):
        vals = p_pos[gather2[c]] * (coefC[c] / s2)[:, :, None]
        im = {
            "slots": np.ascontiguousarray(
                vals.astype(E3M4).transpose(1, 0, 2)),
            "identD": identM,
            "scl": np.full((BLK, 1), s2, dtype=np.float32),
        }
        if has_bias:
            im["biasT"] = np.broadcast_to(
                np.tile(bias_cat, SB)[None, :], (BLK, SB * 64)
            ).astype(np.float32).copy()
        in_maps_C.append(im)
    res_C = _run(nc_C, in_maps_C)

    # ---- unshard (C output is node-major [128, NBLK*64]) ----
    out = np.empty((N, 64), dtype=np.float32)
    for c in range(NCORES):
        _, _, order, _, _ = cores[c]
        o = res_C[c]["outT"].reshape(BLK, NBLK, 64).transpose(1, 0, 2)
        out[c * NPC + order] = o.reshape(NPCP, 64)[:NPC]
    return out[:, :COUT], out[:, COUT:]
